# revision 19
# baseline (speedup 1.0000x reference)
"""Trainium2 Bass kernel for nn_MultiHeadAttention_79465484911033.

Sharding: 8 cores = 2 batches x 4 head-groups (4 heads each of 16).
Each core: QKV projection for its heads (column-parallel), RoPE
(spatial+temporal angles composed into one rotation), causal attention,
swish, and a row-parallel partial output projection. Host sums the 4
partials per batch and adds b_out.

Device layout notes:
- x is passed pre-transposed (xt [H, L]) so the contraction dim is on
  partitions for every matmul.
- q/k are produced transposed ([chan, L]); v natural ([L, chan]).
- RoPE: q_rot = q*cos + (P@q)*sin where P is a pair-swap/sign matrix
  applied on the tensor engine; cos/sin are host-precomputed tables
  with spatial+temporal angles summed (rotations compose).
- Scores are computed transposed (s_T[k, q]) so softmax(exp-only, no
  max subtraction -- scores are bounded) feeds attn@v with p_T as the
  stationary operand; a 2.0-column appended to v yields 2*rowsum in
  the same matmul. swish(o/S) = o*(1+tanh(o/(2S)))/(2S) uses Tanh,
  which shares the Exp activation-table set (no table switches).
- Matmuls run in fp32r (full rate at N>=256); attn@v runs in bf16.
"""

import sys

for _p in ("/opt/trn_rl_repo", "/root/.axon_site/_ro/trn_rl_repo"):
    if _p not in sys.path:
        sys.path.append(_p)

import numpy as np
import ml_dtypes

import concourse.bass as bass
import concourse.mybir as mybir
import concourse.tile as tile
from concourse import bacc
from concourse.bass_utils import run_bass_kernel_spmd

F32 = mybir.dt.float32
F32R = mybir.dt.float32r
BF16 = mybir.dt.bfloat16
AF = mybir.ActivationFunctionType
ALU = mybir.AluOpType

B, L, H = 2, 2048, 1024
NH, HD = 16, 64
NT, LS, L1D = 8, 256, 16
N_CORES = 8
HPC = 4               # heads per core
NKC = H // 128        # 8 contraction chunks
NL = L // 128         # 16 L chunks of 128
NLQ = L // 512        # 4 L tiles of 512

_CACHE = {}


def _build():
    nc = bacc.Bacc("TRN2", target_bir_lowering=False, debug=False,
                   enable_asserts=True, num_devices=N_CORES)

    xt_d = nc.dram_tensor("xt", [H, L], F32R, kind="ExternalInput")
    wqk_d = nc.dram_tensor("wqk", [H, 512], F32R, kind="ExternalInput")
    bqk_d = nc.dram_tensor("bqk", [128, 4], F32, kind="ExternalInput")
    wv_d = nc.dram_tensor("wv", [H, 256], F32R, kind="ExternalInput")
    bv_d = nc.dram_tensor("bv", [1, 256], F32R, kind="ExternalInput")
    cos_d = nc.dram_tensor("cosrep", [128, L], F32, kind="ExternalInput")
    sin_d = nc.dram_tensor("sinrep", [128, L], F32, kind="ExternalInput")
    pt_d = nc.dram_tensor("ptmat", [128, 128], F32R, kind="ExternalInput")
    tri_d = nc.dram_tensor("tri", [128, 128], BF16, kind="ExternalInput")
    id_d = nc.dram_tensor("ident", [128, 128], F32R, kind="ExternalInput")
    wo_d = nc.dram_tensor("woT", [256, 1024], F32R, kind="ExternalInput")
    ones_d = nc.dram_tensor("ones1", [1, 128], F32R, kind="ExternalInput")
    out_d = nc.dram_tensor("out", [L, H], F32, kind="ExternalOutput")

    with tile.TileContext(nc) as tc:
        with (
            tc.tile_pool(name="const", bufs=1) as cpool,
            tc.tile_pool(name="xt", bufs=1) as xpool,
            tc.tile_pool(name="w", bufs=1) as wpool,
            tc.tile_pool(name="qk", bufs=1) as qkpool,
            tc.tile_pool(name="v", bufs=1) as vpool,
            tc.tile_pool(name="work", bufs=1) as work,
            tc.tile_pool(name="pt", bufs=13) as ptpool,
            tc.tile_pool(name="rec", bufs=8) as rpool,
            tc.tile_pool(name="ost", bufs=2) as ostpool,
            tc.tile_pool(name="ps", bufs=1, space="PSUM") as psum,
        ):
            # ---- constants / weights to SBUF ----
            # DMA order = first-needed order.
            cos_t = cpool.tile([128, L], F32, tag="cos")
            sin_t = cpool.tile([128, L], F32, tag="sin")
            ptm_t = cpool.tile([128, 128], F32R, tag="ptm")
            tri_t = cpool.tile([128, 128], BF16, tag="tri")
            id_t = cpool.tile([128, 128], F32R, tag="id")
            bqk_t = cpool.tile([128, 4], F32, tag="bqk")
            bv_t = cpool.tile([1, 256], F32R, tag="bv")
            ones_t = cpool.tile([1, 128], F32R, tag="ones")

            wqk_t = []
            for k in range(NKC):
                t = wpool.tile([128, 512], F32R, tag=f"wqk{k}", name=f"wqk{k}")
                nc.sync.dma_start(t[:], wqk_d[k * 128:(k + 1) * 128, :])
                wqk_t.append(t)
            wv_t = []
            for k in range(NKC):
                t = wpool.tile([128, 256], F32R, tag=f"wv{k}", name=f"wv{k}")
                nc.sync.dma_start(t[:], wv_d[k * 128:(k + 1) * 128, :])
                wv_t.append(t)
            nc.sync.dma_start(bv_t[:], bv_d[:])
            nc.sync.dma_start(ones_t[:], ones_d[:])
            xt_t = []
            for k in range(NKC):
                t = xpool.tile([128, L], F32R, tag=f"xt{k}", name=f"xt{k}")
                nc.sync.dma_start(t[:], xt_d[k * 128:(k + 1) * 128, :])
                xt_t.append(t)
            nc.sync.dma_start(cos_t[:], cos_d[:])
            nc.sync.dma_start(sin_t[:], sin_d[:])
            nc.sync.dma_start(ptm_t[:], pt_d[:])
            nc.sync.dma_start(bqk_t[:], bqk_d[:])
            nc.sync.dma_start(tri_t[:], tri_d[:])
            nc.sync.dma_start(id_t[:], id_d[:])
            wo_t = []
            for g in range(2):
                t = wpool.tile([128, 1024], F32R, tag=f"wo{g}", name=f"wo{g}")
                nc.sync.dma_start(t[:], wo_d[g * 128:(g + 1) * 128, :])
                wo_t.append(t)

            # ---- PE warm-up: dense dummy matmuls while input DMAs land ----
            # Keeps the HAM activity window busy from t~1us so the PE ramps
            # to 2.4 GHz before the real work; results go to a dead DRAM
            # scratch so DCE keeps them.
            warm_dram = nc.dram_tensor("warm_scratch", [128, 128], F32,
                                       kind="Internal")
            wps = psum.tile([128, 512], F32, tag="o", bufs=4, name="warm_ps")
            NWARM = 80
            for w in range(NWARM):
                nc.tensor.matmul(wps[:], wqk_t[0][:, 0:128], wqk_t[0][:],
                                 start=(w == 0), stop=(w == NWARM - 1))
            wsb = work.tile([128, 128], F32, tag="oT", bufs=2, name="warm_sb")
            nc.vector.tensor_copy(wsb[:], wps[:, 0:128])
            nc.sync.dma_start(warm_dram[:], wsb[:])

            # ---- phase A: v projection (natural) + bias, bf16, 2.0 col ----
            v_t = []
            for l in range(NL):
                t = vpool.tile([128, 4 * 65], BF16, tag=f"v{l}", name=f"v{l}")
                for hi in range(HPC):
                    nc.vector.memset(t[:, hi * 65 + 64:hi * 65 + 65], 2.0)
                v_t.append(t)
            for l in range(NL):
                ps = psum.tile([128, 256], F32, tag="sc2", bufs=2,
                               name=f"psv{l}")
                for k in range(NKC):
                    nc.tensor.matmul(
                        ps[:], xt_t[k][:, bass.ts(l, 128)], wv_t[k][:],
                        start=(k == 0), stop=False)
                nc.tensor.matmul(ps[:], ones_t[:], bv_t[:],
                                 start=False, stop=True)
                for hi in range(HPC):
                    nc.vector.tensor_copy(
                        v_t[l][:, hi * 65:hi * 65 + 64],
                        ps[:, bass.ts(hi, 64)])

            # ---- phase B: q/k projection (transposed) + bias + RoPE ----
            # qkrot[m]: [128 chans, L]; m 0,1 = q (heads 0,1 | 2,3),
            # m 2,3 = k likewise.
            qkrot = []
            for m in range(4):
                t = qkpool.tile([128, L], BF16, tag=f"qkr{m}", name=f"qkr{m}")
                qkrot.append(t)

            for m in (0, 2, 1, 3):
                for n in range(NLQ):
                    sl = bass.ts(n, 512)
                    qkb = work.tile([128, 512], F32R, tag="qkb", bufs=3,
                                    name=f"qkb{m}_{n}")
                    ps = psum.tile([128, 512], F32, tag="sc2", bufs=2,
                                   name=f"psqk{m}_{n}")
                    for k in range(NKC):
                        nc.tensor.matmul(
                            ps[:], wqk_t[k][:, bass.ts(m, 128)], xt_t[k][:, sl],
                            start=(k == 0), stop=(k == NKC - 1))
                    # bias add (per-partition) psum -> sbuf
                    nc.scalar.add(qkb[:], ps[:], bqk_t[:, m:m + 1])
                    # rope shuffle: sh = P @ qkb
                    sh = psum.tile([128, 512], F32, tag="sc2", bufs=2,
                                   name=f"pssh{m}_{n}")
                    nc.tensor.matmul(sh[:], ptm_t[:], qkb[:],
                                     start=True, stop=True)
                    # rot = qkb*cos + sh*sin (bf16 out: halves scores
                    # weight-load via FWL and the attention SBUF footprint)
                    nc.vector.tensor_mul(qkrot[m][:, sl],
                                         qkb[:].bitcast(F32), cos_t[:, sl])
                    nc.vector.tensor_mul(qkb[:], sh[:], sin_t[:, sl])
                    nc.vector.tensor_add(qkrot[m][:, sl],
                                         qkrot[m][:, sl], qkb[:].bitcast(F32))

            # ---- phase C: attention (i outer, head inner) + fused oproj ----
            # o_sw[l] [128, 256] f32r: natural-layout swish(attention) output,
            # aliased onto the (dead after phase B) xt tiles' SBUF.
            o_sw = []
            for l in range(NL):
                base = (l % 8) * 256
                o_sw.append(xt_t[l // 8][:, base:base + 256])

            pending = []

            def flush(kq):
                for _ in range(min(kq, len(pending))):
                    pending.pop(0)()

            def queue_head(hi, i, pts):
                # j-major attn@v + swish for one head, as deferred emissions
                o_ps = [psum.tile([128, 65], F32, tag="o", bufs=4,
                                  name=f"ops{hi}_{i}_{s}") for s in range(4)]

                def av(j, s):
                    def emit():
                        tile, off = pts[j]
                        nc.tensor.matmul(
                            o_ps[s][:],
                            tile[:, off + s * 128:off + (s + 1) * 128],
                            v_t[j][:, hi * 65:hi * 65 + 65],
                            start=(j == 0), stop=(j == 4 * i + s))
                    return emit

                def silu(s):
                    def emit():
                        # swish(o/S) = o*(1+tanh(o/(2S)))/(2S); col64 = 2S
                        rec = rpool.tile([128, 1], F32, tag="rec",
                                         name=f"rec{hi}_{i}_{s}")
                        nc.vector.reciprocal(rec[:], o_ps[s][:, 64:65])
                        th = rpool.tile([128, 64], F32, tag="th", bufs=3,
                                        name=f"th{hi}_{i}_{s}")
                        nc.scalar.activation(th[:], o_ps[s][:, 0:64],
                                             AF.Tanh, scale=rec[:])
                        t_n = rpool.tile([128, 64], F32, tag="tn", bufs=3,
                                         name=f"tn{hi}_{i}_{s}")
                        nc.vector.tensor_scalar_mul(t_n[:], o_ps[s][:, 0:64],
                                                    rec[:])
                        nc.vector.scalar_tensor_tensor(
                            o_sw[4 * i + s][:, bass.ts(hi, 64)],
                            th[:], 1.0, t_n[:],
                            op0=ALU.add, op1=ALU.mult)
                    return emit

                for j in range(4 * i + 4):
                    d = j - 4 * i
                    for s in range(max(0, d), 4):
                        pending.append(av(j, s))
                        if j == 4 * i + s:
                            pending.append(silu(s))

            def oproj_block(i):
                # transpose + output projection for L-chunks of q-tile i
                for l in range(4 * i, 4 * i + 4):
                    oTl = work.tile([128, 256], F32R, tag="oT", bufs=2,
                                    name=f"oT{l}")
                    for g in range(2):
                        pst = psum.tile([128, 128], F32R, tag="o", bufs=4,
                                        name=f"pst{l}_{g}")
                        nc.tensor.transpose(pst[:],
                                            o_sw[l][:, bass.ts(g, 128)],
                                            id_t[:])
                        nc.vector.tensor_copy(oTl[:, bass.ts(g, 128)], pst[:])
                    for n in range(2):
                        ps = psum.tile([128, 512], F32, tag="sc2", bufs=2,
                                       name=f"pso{l}_{n}")
                        for g in range(2):
                            nc.tensor.matmul(
                                ps[:], oTl[:, bass.ts(g, 128)],
                                wo_t[g][:, bass.ts(n, 512)],
                                start=(g == 0), stop=(g == 1))
                        ost = ostpool.tile([128, 512], F32, tag="ost")
                        nc.vector.tensor_copy(ost[:], ps[:])
                        nc.sync.dma_start(
                            out_d[bass.ts(l, 128), bass.ts(n, 512)], ost[:])

            for i in range(NLQ):
                nj = 4 * i + 4
                for hi in range(HPC):
                    q_sl = qkrot[hi // 2][(hi % 2) * 64:(hi % 2) * 64 + 64, :]
                    k_sl = qkrot[2 + hi // 2][(hi % 2) * 64:(hi % 2) * 64 + 64, :]
                    per_j = (len(pending) + nj - 1) // nj + 1
                    pts = []
                    for jp in range(nj // 2):
                        j0, j1 = 2 * jp, 2 * jp + 1
                        d0, d1 = j0 - 4 * i, j1 - 4 * i
                        ss = psum.tile([128, 1024], F32, tag="sc2", bufs=2,
                                       name=f"ss{hi}_{i}_{jp}")
                        nc.tensor.matmul(
                            ss[:, 0:512], k_sl[:, bass.ts(j0, 128)],
                            q_sl[:, bass.ts(i, 512)], start=True, stop=True)
                        nc.tensor.matmul(
                            ss[:, 512:1024], k_sl[:, bass.ts(j1, 128)],
                            q_sl[:, bass.ts(i, 512)], start=True, stop=True)
                        pt = ptpool.tile([128, 1024], BF16, tag="pt",
                                         name=f"pt{hi}_{i}_{jp}")
                        vf0 = max(0, d0) * 128
                        nc.scalar.activation(pt[:, vf0:1024], ss[:, vf0:1024],
                                             AF.Exp, scale=0.125)
                        if d0 >= 0:
                            nc.vector.tensor_mul(pt[:, vf0:vf0 + 128],
                                                 pt[:, vf0:vf0 + 128],
                                                 tri_t[:])
                        if d1 >= 0:
                            vb = 512 + d1 * 128
                            nc.vector.tensor_mul(pt[:, vb:vb + 128],
                                                 pt[:, vb:vb + 128], tri_t[:])
                        pts.append((pt, 0))
                        pts.append((pt, 512))
                        flush(2 * per_j)
                    queue_head(hi, i, pts)
                # drain: oproj below needs every head's swish done
                flush(len(pending))
                oproj_block(i)

    nc.compile()
    return nc


def _rope_tables():
    f2 = 1.0 / (10000.0 ** (np.arange(0, HD, 4, dtype=np.float64)[:HD // 4] / HD))
    s = np.arange(LS, dtype=np.float64)
    ang_s = np.zeros((LS, HD // 2), dtype=np.float64)
    ang_s[:, :HD // 4] = np.outer(s % L1D, f2)
    ang_s[:, HD // 4:] = np.outer(s // L1D, f2)
    f1 = 1.0 / (10000.0 ** (np.arange(0, HD, 2, dtype=np.float64) / HD))
    ang_t = np.outer(np.arange(NT, dtype=np.float64), f1)
    l = np.arange(L)
    ang = ang_s[l % LS] + ang_t[l // LS]        # [L, 32]
    pair = (np.arange(128) % HD) // 2           # [128] -> pair index
    cosrep = np.cos(ang).T[pair].astype(np.float32)  # [128, L]
    sinrep = np.sin(ang).T[pair].astype(np.float32)
    return np.ascontiguousarray(cosrep), np.ascontiguousarray(sinrep)


def _const_inputs():
    cosrep, sinrep = _rope_tables()
    ptmat = np.zeros((128, 128), dtype=np.float32)
    for i in range(64):
        ptmat[2 * i + 1, 2 * i] = -1.0   # shuffle[2i]   = -q[2i+1]
        ptmat[2 * i, 2 * i + 1] = 1.0    # shuffle[2i+1] = +q[2i]
    r = np.arange(128)
    tri = (r[None, :] >= r[:, None]).astype(ml_dtypes.bfloat16)  # [k, q]
    ident = np.eye(128, dtype=np.float32)
    return cosrep, sinrep, ptmat, tri, ident


def _make_in_maps(inp):
    x = np.asarray(inp["x"], dtype=np.float32)
    w_qkv = np.asarray(inp["w_qkv"], dtype=np.float32)
    b_qkv = np.asarray(inp["b_qkv"], dtype=np.float32)
    w_out = np.asarray(inp["w_out"], dtype=np.float32)
    if "consts" not in _CACHE:
        _CACHE["consts"] = _const_inputs()
    cosrep, sinrep, ptmat, tri, ident = _CACHE["consts"]
    in_maps = []
    for c in range(N_CORES):
        b = c // 4
        heads = [4 * (c % 4) + i for i in range(HPC)]
        qrows = [h * 192 + j for h in heads for j in range(64)]
        krows = [h * 192 + 64 + j for h in heads for j in range(64)]
        vrows = [h * 192 + 128 + j for h in heads for j in range(64)]
        ocols = [h * 64 + j for h in heads for j in range(64)]
        wqk = np.ascontiguousarray(w_qkv[qrows + krows, :].T)
        bqk = np.ascontiguousarray(b_qkv[qrows + krows].reshape(4, 128).T)
        wv = np.ascontiguousarray(w_qkv[vrows, :].T)
        bv = np.ascontiguousarray(b_qkv[vrows].reshape(1, 256))
        woT = np.ascontiguousarray(w_out[:, ocols].T)
        xt = np.ascontiguousarray(x[b].T)
        in_maps.append({
            "xt": xt, "wqk": wqk, "bqk": bqk, "wv": wv, "bv": bv,
            "cosrep": cosrep, "sinrep": sinrep, "ptmat": ptmat,
            "tri": tri, "ident": ident, "woT": woT,
            "ones1": np.ones((1, 128), dtype=np.float32),
        })
    return in_maps


def kernel(x, w_qkv, b_qkv, w_out, b_out):
    b_out = np.asarray(b_out, dtype=np.float32)
    if "nc" not in _CACHE:
        _CACHE["nc"] = _build()
    nc = _CACHE["nc"]
    in_maps = _make_in_maps({"x": x, "w_qkv": w_qkv, "b_qkv": b_qkv,
                             "w_out": w_out})

    res = run_bass_kernel_spmd(nc, in_maps, core_ids=list(range(N_CORES)))

    out = np.zeros((B, L, H), dtype=np.float32)
    for c in range(N_CORES):
        out[c // 4] += res.results[c]["out"]
    out += b_out[None, None, :]
    return out
